# revision 38
# baseline (speedup 1.0000x reference)
"""Causal self-attention kernel for TRN2 (8 NeuronCores, Bass/Tile).

Problem: B=8, T=1024, C=768, H=12, HD=64.
  qkv = x @ W_attn + b_attn ; causal softmax attention ; y = att_out @ W_proj + b_proj

Sharding: pure data-parallel over batch - core b computes batch element b.

Per-core dataflow (all matmuls bf16 on the PE):
  xT   [768,1024]  host-pre-transposed, loaded with 2 contiguous DMAs
  qkT  [1536,1024] : qkT[c',t] = sum_c W[c,c'] xT[c,t]
  V    [1024,768]  : V[t,c'] = sum_c xT[c,t] W_v[c,c']   (per-head Vp tiles with
                     a trailing ones column -> PV matmul also produces Z row)
  per head h, i-block (512 cols):
     ST[j,i] = kT^T q  (K=64, two heads packed in PE row groups 0/64)
     exp(0.125*ST) on ScalarE -> bf16 est; tri-mask diagonal 128x128 (DVE mult)
     OT'[0:64,:] = unnormalized out (transposed), OT'[64,:] = Z, PSUM accumulate
     ATn[c,t] = OT'[0:64]/Z  (DVE mult with 1/Z broadcast via gpsimd)
  y[t,:] = ATn^T-contraction with W_proj   (bf16 out, host casts to fp32)

Scheduling: emission is software-pipelined.  Within an attention the PV of
step jc-1 is emitted after the ST of step jc, and independent matmul work
(v/qk projections, output projection) is pumped between attention steps via
filler generators so the PE queue never head-of-line blocks on the exp chain.
The last attention uses per-quarter PSUM accumulation stops so each output
quarter normalizes early and feeds its proj tile without draining the tail.

Zero-bias fast path: when b_attn == 0 and b_proj == 0 (the spec fill), the
program skips all bias work; otherwise a bias-enabled program is compiled.
"""

import numpy as np

import concourse.bass as bass
import concourse.mybir as mybir
import concourse.tile as tile
from concourse import bacc
from concourse.bass_utils import run_bass_kernel_spmd

F32 = mybir.dt.float32
BF16 = mybir.dt.bfloat16
AF = mybir.ActivationFunctionType
ALU = mybir.AluOpType

T, C, H, HD = 1024, 768, 12, 64
NCORES = 8
CC = C // 128          # 6 contraction chunks
TP = T // 128          # 8 t-chunks of 128
TB = T // 512          # 2 t-blocks of 512
QKCP = 2 * C // 128    # 12 qkT partition tiles
SCALE = 1.0 / 8.0      # 1/sqrt(64)

_PROGRAM_CACHE = {}


def build_program(with_bias):
    nc = bacc.Bacc(
        "TRN2", target_bir_lowering=False, debug=False, enable_partition_id=False
    )

    # x arrives pre-transposed from the host: xT[c, t] = x[t, c]
    x_d = nc.dram_tensor("xT", [C, T], BF16, kind="ExternalInput").ap()
    wa_d = nc.dram_tensor("W_attn", [C, 3 * C], BF16, kind="ExternalInput").ap()
    ba_d = nc.dram_tensor("b_attn", [1, 3 * C], BF16, kind="ExternalInput").ap()
    wp_d = nc.dram_tensor("W_proj", [C, C], BF16, kind="ExternalInput").ap()
    bp_d = nc.dram_tensor("b_proj", [1, C], BF16, kind="ExternalInput").ap()
    y_d = nc.dram_tensor("y", [T, C], BF16, kind="ExternalOutput").ap()

    with tile.TileContext(nc) as tc:
        _emit(nc, tc, x_d, wa_d, ba_d, wp_d, bp_d, y_d, with_bias)
    nc.compile()
    return nc


def _pump(g, n=1):
    if g is None:
        return
    for _ in range(n):
        try:
            next(g)
        except StopIteration:
            return


def _drain(g):
    if g is None:
        return
    for _ in g:
        pass


def _emit(nc, tc, x_d, wa_d, ba_d, wp_d, bp_d, y_d, with_bias):
    from contextlib import ExitStack
    from itertools import chain

    ctx = ExitStack()
    with ctx:
        const_pool = ctx.enter_context(tc.tile_pool(name="consts", bufs=1))
        # ps_work holds the merged [128,1024] ST tiles (2 banks each);
        # ps_acc holds 1-bank accumulation tiles (qk/v/y) + OT' tiles.
        ps_work = ctx.enter_context(tc.tile_pool(name="ps_work", bufs=2, space="PSUM"))
        ps_acc = ctx.enter_context(tc.tile_pool(name="ps_acc", bufs=2, space="PSUM"))

        # ---- inputs: merged tiles, need-ordered 3D DMAs on both rings ----
        # scalar ring: xT h0, qk first halves, xT h1, qk second halves
        # sync ring:   v-part (vc-major), then W_proj
        xt_pool = ctx.enter_context(tc.tile_pool(name="xt", bufs=1))
        xT = []
        for cc in range(CC):
            t_ = xt_pool.tile([128, T], BF16, name=f"xT_{cc}", tag=f"xT{cc}")
            xT.append(t_)

        def xts(cc, t0, w):
            return xT[cc][:, t0 : t0 + w]

        # W_all layout: [v vc-major 2*2304 | qh0 2304 | kh0 2304 | qh1 2304
        #                | kh1 2304] where qh/kh h holds head-pairs 3h..3h+2
        w_pool = ctx.enter_context(tc.tile_pool(name="w", bufs=1))
        W_all = w_pool.tile([128, CC * 3 * C], BF16, name="W_all", tag="W")

        def wv(cc, vc):
            base = vc * 2304 + cc * 384
            return W_all[:, base : base + 384]

        def wqk(cc, cp):
            part, hp = cp // CC, cp % CC
            half, hh = hp // 3, hp % 3
            base = 4608 + half * 4608 + part * 2304 + cc * 384 + hh * 128
            return W_all[:, base : base + 128]

        def _w_dma(queue, dst_base, src_col0, width):
            queue.dma_start(
                W_all[:, dst_base : dst_base + CC * width].rearrange(
                    "p (cc w) -> p cc w", cc=CC
                ),
                wa_d[:, src_col0 : src_col0 + width].rearrange(
                    "(cc p) w -> p cc w", p=128
                ),
            )

        # emission order defines ring order
        for cc in range(CC):
            nc.scalar.dma_start(
                xT[cc][:, 0:512], x_d[cc * 128 : (cc + 1) * 128, 0:512]
            )
        _w_dma(nc.sync, 0, 1536, 384)      # v vc=0
        _w_dma(nc.scalar, 4608, 0, 384)    # q half 0 (pairs 0-2)
        _w_dma(nc.scalar, 6912, 768, 384)  # k half 0
        _w_dma(nc.sync, 2304, 1920, 384)   # v vc=1
        for cc in range(CC):
            nc.scalar.dma_start(
                xT[cc][:, 512:1024], x_d[cc * 128 : (cc + 1) * 128, 512:1024]
            )
        _w_dma(nc.scalar, 9216, 384, 384)   # q half 1 (pairs 3-5)
        _w_dma(nc.scalar, 11520, 1152, 384) # k half 1

        wp_pool = ctx.enter_context(tc.tile_pool(name="wp", bufs=1))
        Wp_all = wp_pool.tile([128, CC * C], BF16, name="Wp_all", tag="Wp")
        nc.sync.dma_start(
            Wp_all.rearrange("p (cc w) -> p cc w", cc=CC),
            wp_d.rearrange("(cc p) w -> p cc w", p=128),
        )

        def wpj(cc, oc):
            return Wp_all[:, cc * C + oc * 384 : cc * C + (oc + 1) * 384]

        # ---- constants -------------------------------------------------
        # tri[j, i] = 1.0 if j <= i else 0.0   (keep lower-causal in [j,i] layout)
        tri_f32 = const_pool.tile([128, 128], F32, name="tri_f32")
        nc.gpsimd.memset(tri_f32[:], 1.0)
        nc.gpsimd.affine_select(
            out=tri_f32[:], in_=tri_f32[:], compare_op=ALU.is_ge, fill=0.0,
            base=0, pattern=[[1, 128]], channel_multiplier=-1,
        )
        tri = const_pool.tile([128, 128], BF16, name="tri")
        nc.vector.tensor_copy(tri[:], tri_f32[:])
        ones32 = const_pool.tile([128, 16], F32, name="ones32")
        nc.gpsimd.memset(ones32[:], 1.0)

        # warm the exp table set early (hidden under input DMA)
        expwarm = const_pool.tile([1, 1], F32, name="expwarm")
        nc.scalar.activation(expwarm[:], ones32[0:1, 0:1], AF.Exp)

        if with_bias:
            ones_row = const_pool.tile([1, 512], BF16, name="ones_row")
            nc.gpsimd.memset(ones_row[:], 1.0)
            ba_sb = const_pool.tile([1, 3 * C], BF16, name="ba_sb")
            nc.scalar.dma_start(ba_sb[:], ba_d[:, :])
            bp_sb = const_pool.tile([1, C], BF16, name="bp_sb")
            nc.scalar.dma_start(bp_sb[:], bp_d[:, :])
            # column layout of b_attn qk-part for per-partition bias add
            ba_col = const_pool.tile([128, QKCP], F32, name="ba_col")
            ba_colb = const_pool.tile([128, QKCP], BF16, name="ba_colb")
            nc.scalar.dma_start(
                ba_colb[:],
                ba_d[:, 0 : QKCP * 128].rearrange("a (cp p) -> (a p) cp", p=128),
            )
            nc.vector.tensor_copy(ba_col[:], ba_colb[:])

        # ---- phase B: Vp then qkT (attention needs all Vp) -------------
        vp_pool = ctx.enter_context(tc.tile_pool(name="vp", bufs=1))
        Vp = []
        for tp in range(TP):
            t_ = vp_pool.tile([128, H * 65], BF16, name=f"Vp_{tp}", tag=f"Vp{tp}")
            Vp.append(t_)
            nc.vector.tensor_copy(
                t_.rearrange("p (h e) -> p h e", e=65)[:, :, 64:65],
                ones32[:, 0:H].rearrange("p (h e) -> p h e", e=1),
            )

        def v_chunk_gen(tp):
            # each yield leaves a consistent state: the chain's Vp copy is
            # emitted before the final yield so consumers can follow safely
            for vc in range(2):  # v cols [1536+384*vc : 1536+384*(vc+1)]
                pv = ps_acc.tile([128, 384], F32, name=f"ps_v_{vc}_{tp}", tag="acc")
                for cc in range(CC):
                    nc.tensor.matmul(
                        pv[:],
                        xts(cc, tp * 128, 128),
                        wv(cc, vc),
                        start=(cc == 0),
                        stop=(cc == CC - 1) and not with_bias,
                    )
                    if cc < CC - 1:
                        yield
                if with_bias:
                    nc.tensor.matmul(
                        pv[:],
                        ones_row[:, 0:128],
                        ba_sb[:, 1536 + vc * 384 : 1536 + (vc + 1) * 384],
                        start=False,
                        stop=True,
                    )
                # one strided copy drops all 6 heads into their Vp slots;
                # ScalarE carries it (idle during the projection ramp)
                nc.scalar.copy(
                    Vp[tp].rearrange("p (h e) -> p h e", e=65)[
                        :, 6 * vc : 6 * vc + 6, 0:64
                    ],
                    pv.rearrange("p (h e) -> p h e", e=64)[:, :, :],
                )
                yield

        qkt_pool = ctx.enter_context(tc.tile_pool(name="qkt", bufs=1))
        qkT = []
        for cp in range(QKCP):
            t_ = qkt_pool.tile([128, T], BF16, name=f"qkT_{cp}", tag=f"qkT{cp}")
            qkT.append(t_)

        def qk_pair_gen(hp):
            # copy emitted before the chain's final yield (see v_chunk_gen)
            for tb in range(TB):
                for cp in (hp, 6 + hp):
                    pq = ps_acc.tile([128, 512], F32, name=f"ps_qk_{cp}_{tb}", tag="acc")
                    for cc in range(CC):
                        nc.tensor.matmul(
                            pq[:],
                            wqk(cc, cp),
                            xts(cc, tb * 512, 512),
                            start=(cc == 0),
                            stop=(cc == CC - 1),
                        )
                        if cc < CC - 1:
                            yield
                    if with_bias:
                        # b_attn[c'] folded in as a per-partition scalar add
                        nc.vector.tensor_scalar_add(
                            qkT[cp][:, tb * 512 : (tb + 1) * 512],
                            pq[:],
                            ba_col[:, cp : cp + 1],
                        )
                    else:
                        nc.vector.tensor_copy(
                            qkT[cp][:, tb * 512 : (tb + 1) * 512], pq[:]
                        )
                    yield

        # ---- phase C/D: attention (ib-major) interleaved with proj -----
        atn_pool = ctx.enter_context(tc.tile_pool(name="atn", bufs=1))
        ATn = []
        for cp in range(CC):
            t_ = atn_pool.tile([128, T], BF16, name=f"ATn_{cp}", tag=f"ATn{cp}")
            ATn.append(t_)

        est_pool = ctx.enter_context(tc.tile_pool(name="est", bufs=10))
        nrm_pool = ctx.enter_context(tc.tile_pool(name="nrm", bufs=4))
        y_pool = ctx.enter_context(tc.tile_pool(name="ysb", bufs=2))

        def norm(hp, ib, po, i0, w):
            """Normalize OT'->ATn for columns [i0, i0+w) of i-block ib.

            Batched over both heads: one scatter/recip/gather/broadcast chain.
            """
            key = f"{hp}_{ib}_{i0}"
            otu = nrm_pool.tile([65, 2 * w], BF16, name=f"otu_{key}", tag="otu")
            for s in range(2):
                nc.vector.tensor_copy(otu[:, s * w : (s + 1) * w], po[s][:, i0 : i0 + w])
            nw = 2 * w // 128
            zs = nrm_pool.tile([128, nw], BF16, name=f"zs_{key}", tag="zs")
            nc.sync.dma_start(zs[:], otu[64:65, :])
            zr = nrm_pool.tile([128, nw], F32, name=f"zr_{key}", tag="zr")
            nc.vector.reciprocal(zr[:], zs[:])
            zrb = nrm_pool.tile([128, nw], BF16, name=f"zrb_{key}", tag="zrb")
            nc.vector.tensor_copy(zrb[:], zr[:])
            zinv = nrm_pool.tile([1, 2 * w], BF16, name=f"zinv_{key}", tag="zinv")
            nc.sync.dma_start(zinv[:], zrb[:])
            zb = nrm_pool.tile([64, 2 * w], BF16, name=f"zb_{key}", tag="zb")
            nc.gpsimd.partition_broadcast(zb[:], zinv[:])
            for s in range(2):
                nc.vector.tensor_tensor(
                    ATn[hp][64 * s : 64 * s + 64, ib * 512 + i0 : ib * 512 + i0 + w],
                    otu[0:64, s * w : (s + 1) * w],
                    zb[:, s * w : (s + 1) * w],
                    op=ALU.mult,
                )

        def attention(hp, ib, quartered=False, filler=None, drain_filler=True):
            qt = qkT[hp]
            kt = qkT[6 + hp]
            po = {}
            for s in range(2):  # head 2*hp + s
                po[s] = ps_acc.tile([65, 512], F32, name=f"ps_ot_{hp}_{ib}_{s}", tag="ot", bufs=2)
            njc = 4 * (ib + 1)
            ests = {}

            def emit_st(jc):
                r = jc - 4 * ib
                col0 = max(r, 0) * 128
                # merged pair tile: head A in cols [0:512], head B in [512:1024]
                pst = ps_work.tile([128, 1024], F32, name=f"ps_st_{hp}_{ib}_{jc}", tag="ps")
                for s in range(2):
                    r0 = 64 * s
                    # row-packed pair: s=0 uses PE rows 0-63, s=1 rows 64-127
                    nc.tensor.matmul(
                        pst[:, 512 * s + col0 : 512 * s + 512],
                        kt[r0 : r0 + 64, jc * 128 : (jc + 1) * 128],
                        qt[r0 : r0 + 64, ib * 512 + col0 : (ib + 1) * 512],
                        start=True,
                        stop=True,
                    )
                est = est_pool.tile([128, 1024], BF16, name=f"est_{hp}_{ib}_{jc}", tag="est")
                ests[jc] = est
                nc.scalar.activation(
                    est.rearrange("p (a f) -> p a f", a=2)[:, :, col0:512],
                    pst.rearrange("p (a f) -> p a f", a=2)[:, :, col0:512],
                    AF.Exp,
                    scale=SCALE,
                )
                if r >= 0:
                    for s in range(2):
                        # mask the diagonal 128x128 sub-block (multiplicative)
                        nc.vector.tensor_tensor(
                            est[:, 512 * s + col0 : 512 * s + col0 + 128],
                            est[:, 512 * s + col0 : 512 * s + col0 + 128],
                            tri[:],
                            op=ALU.mult,
                        )

            def emit_pv(jc):
                r = jc - 4 * ib
                col0 = max(r, 0) * 128
                est = ests.pop(jc)
                if not quartered:
                    for s in range(2):
                        h = 2 * hp + s
                        nc.tensor.matmul(
                            po[s][:, col0:512],
                            Vp[jc][:, h * 65 : h * 65 + 65],
                            est[:, 512 * s + col0 : 512 * s + 512],
                            start=(jc == 0),
                            stop=(jc == njc - 1),
                        )
                else:
                    # per-quarter accumulation stop: quarter q (cols q*128..)
                    # completes at jc == 4+q, freeing its normalization early.
                    # start only once per PSUM bank (zeroes the whole region).
                    for s in range(2):
                        h = 2 * hp + s
                        for q in range(max(r, 0), 4):
                            nc.tensor.matmul(
                                po[s][:, q * 128 : (q + 1) * 128],
                                Vp[jc][:, h * 65 : h * 65 + 65],
                                est[:, 512 * s + q * 128 : 512 * s + (q + 1) * 128],
                                start=(jc == 0 and q == 0),
                                stop=(jc == 4 + q),
                            )
                    if r >= 0:
                        # quarter r just received its stop -> normalize it
                        norm(hp, ib, po, r * 128, 128)

            # software pipeline: ST(jc) ahead of PV(jc-1), filler in between
            for jc in range(njc):
                emit_st(jc)
                _pump(filler, 1)
                if jc >= 1:
                    emit_pv(jc - 1)
                    _pump(filler, 1)
            emit_pv(njc - 1)
            if not quartered:
                norm(hp, ib, po, 0, 512)
            if drain_filler:
                _drain(filler)

        def proj_gen(tp, cps=range(CC), py_held=None, start=True, stop=True):
            """Emit proj-chain matmuls for t-block tp over contraction tiles
            cps.  With py_held, continues previously-started chains."""
            pys = py_held if py_held is not None else {}
            y_sb = None
            if stop:
                y_sb = y_pool.tile([128, C], BF16, name=f"y_sb_{tp}", tag="y_sb")
            for oc in range(2):
                if oc not in pys:
                    pys[oc] = ps_acc.tile(
                        [128, 384], F32, name=f"ps_y_{tp}_{oc}", tag="acc"
                    )
                py = pys[oc]
                cps_l = list(cps)
                for k, cp in enumerate(cps_l):
                    nc.tensor.matmul(
                        py[:],
                        ATn[cp][:, tp * 128 : (tp + 1) * 128],
                        wpj(cp, oc),
                        start=start and (k == 0),
                        stop=stop and (k == len(cps_l) - 1) and not with_bias,
                    )
                    yield
                if stop and with_bias:
                    nc.tensor.matmul(
                        py[:],
                        ones_row[:, 0:128],
                        bp_sb[:, oc * 384 : (oc + 1) * 384],
                        start=False,
                        stop=True,
                    )
                    yield
                if stop:
                    # copy+DMA per half so the first half streams out while
                    # the second chain finishes
                    nc.vector.tensor_copy(
                        y_sb[:, oc * 384 : (oc + 1) * 384], pys[oc][:]
                    )
                    nc.scalar.dma_start(
                        y_d[tp * 128 : (tp + 1) * 128, oc * 384 : (oc + 1) * 384],
                        y_sb[:, oc * 384 : (oc + 1) * 384],
                    )

        # ---- emission schedule -----------------------------------------
        # Each head pair runs ib=0 then ib=1 back-to-back, fed by the
        # remaining qk/v projection work as fillers.  All eight proj tiles
        # are deferred to the tail, forming a dense PE block that hides the
        # last attention's exp chain and keeps the PE clock warm.
        for tp in range(4):
            _drain(v_chunk_gen(tp))
        _drain(qk_pair_gen(0))
        attention(
            0, 0,
            filler=chain(*[v_chunk_gen(tp) for tp in range(4, 8)]),
        )
        # rolling filler: qk pair hp+1 spreads across attention(hp,0) and
        # (hp,1); its tb=0 half is forced out before attention(hp+1,0) STs
        g = {hp: qk_pair_gen(hp) for hp in range(1, 6)}
        _pump(g[1], 12)
        attention(0, 1, filler=g[1])
        for hp in range(1, 5):
            attention(hp, 0, filler=g[hp + 1], drain_filler=False)
            _pump(g[hp + 1], 12)  # force out the tb=0 remainder
            attention(hp, 1, filler=g[hp + 1])
        # proj(0) cp 0..4 don't touch ATn[5]: they fill attention(5,0)
        py0 = {}
        attention(5, 0, filler=proj_gen(0, range(5), py0, True, False))
        attention(
            5, 1, quartered=True,
            filler=chain(
                proj_gen(0, [5], py0, False, True),
                proj_gen(1), proj_gen(2), proj_gen(3),
            ),
        )
        # tail: per-quarter norms pipelined during the loop feed these chains
        for tp in range(4, 8):
            _drain(proj_gen(tp))


def kernel(x, W_attn, b_attn, W_proj, b_proj, _trace=False, _trace_kwargs=None):
    import ml_dtypes

    bf16 = ml_dtypes.bfloat16
    with_bias = bool(np.any(np.asarray(b_attn)) or np.any(np.asarray(b_proj)))
    x = np.ascontiguousarray(np.asarray(x).astype(bf16))
    W_attn = np.ascontiguousarray(np.asarray(W_attn).astype(bf16))
    b_attn = np.ascontiguousarray(np.asarray(b_attn).astype(bf16)).reshape(1, 3 * C)
    W_proj = np.ascontiguousarray(np.asarray(W_proj).astype(bf16))
    b_proj = np.ascontiguousarray(np.asarray(b_proj).astype(bf16)).reshape(1, C)

    key = f"prog_{with_bias}"
    if key not in _PROGRAM_CACHE:
        _PROGRAM_CACHE[key] = build_program(with_bias)
    nc = _PROGRAM_CACHE[key]

    in_maps = [
        {
            "xT": np.ascontiguousarray(x[b].T),
            "W_attn": W_attn,
            "b_attn": b_attn,
            "W_proj": W_proj,
            "b_proj": b_proj,
        }
        for b in range(NCORES)
    ]
    res = run_bass_kernel_spmd(
        nc,
        in_maps,
        core_ids=list(range(NCORES)),
        trace=_trace,
        **(_trace_kwargs or {}),
    )
    out = np.stack(
        [res.results[b]["y"].astype(np.float32) for b in range(NCORES)], axis=0
    )
    if _trace:
        return out, res
    return out


if __name__ == "__main__":
    rng = np.random.default_rng(0)
    x = rng.standard_normal((NCORES, T, C)).astype(np.float32)
    W_attn = (rng.standard_normal((C, 3 * C)) * 0.02).astype(np.float32)
    b_attn = np.zeros(3 * C, np.float32)
    W_proj = (rng.standard_normal((C, C)) * 0.02).astype(np.float32)
    b_proj = np.zeros(C, np.float32)
    y = kernel(x=x, W_attn=W_attn, b_attn=b_attn, W_proj=W_proj, b_proj=b_proj)
    print("out", y.shape, y.dtype, np.abs(y).max())


# revision 39
# speedup vs baseline: 1.0866x; 1.0866x over previous
"""Causal self-attention kernel for TRN2 (8 NeuronCores, Bass/Tile).

Problem: B=8, T=1024, C=768, H=12, HD=64.
  qkv = x @ W_attn + b_attn ; causal softmax attention ; y = att_out @ W_proj + b_proj

Sharding: pure data-parallel over batch - core b computes batch element b.

Per-core dataflow (all matmuls bf16 on the PE):
  xT   [768,1024]  host-pre-transposed, loaded with 2 contiguous DMAs
  qkT  [1536,1024] : qkT[c',t] = sum_c W[c,c'] xT[c,t]
  V    [1024,768]  : V[t,c'] = sum_c xT[c,t] W_v[c,c']   (per-head Vp tiles with
                     a trailing ones column -> PV matmul also produces Z row)
  per head h, i-block (512 cols):
     ST[j,i] = kT^T q  (K=64, two heads packed in PE row groups 0/64)
     exp(0.125*ST) on ScalarE -> bf16 est; tri-mask diagonal 128x128 (DVE mult)
     OT'[0:64,:] = unnormalized out (transposed), OT'[64,:] = Z, PSUM accumulate
     ATn[c,t] = OT'[0:64]/Z  (DVE mult with 1/Z broadcast via gpsimd)
  y[t,:] = ATn^T-contraction with W_proj   (bf16 out, host casts to fp32)

Scheduling: emission is software-pipelined.  Within an attention the PV of
step jc-1 is emitted after the ST of step jc, and independent matmul work
(v/qk projections, output projection) is pumped between attention steps via
filler generators so the PE queue never head-of-line blocks on the exp chain.
The last attention uses per-quarter PSUM accumulation stops so each output
quarter normalizes early and feeds its proj tile without draining the tail.

Zero-bias fast path: when b_attn == 0 and b_proj == 0 (the spec fill), the
program skips all bias work; otherwise a bias-enabled program is compiled.
"""

import numpy as np

import concourse.bass as bass
import concourse.mybir as mybir
import concourse.tile as tile
from concourse import bacc
from concourse.bass_utils import run_bass_kernel_spmd

F32 = mybir.dt.float32
BF16 = mybir.dt.bfloat16
AF = mybir.ActivationFunctionType
ALU = mybir.AluOpType

T, C, H, HD = 1024, 768, 12, 64
NCORES = 8
CC = C // 128          # 6 contraction chunks
TP = T // 128          # 8 t-chunks of 128
TB = T // 512          # 2 t-blocks of 512
QKCP = 2 * C // 128    # 12 qkT partition tiles
SCALE = 1.0 / 8.0      # 1/sqrt(64)

_PROGRAM_CACHE = {}


def build_program(with_bias):
    nc = bacc.Bacc(
        "TRN2", target_bir_lowering=False, debug=False, enable_partition_id=False
    )

    # x arrives pre-transposed from the host: xT[c, t] = x[t, c]
    x_d = nc.dram_tensor("xT", [C, T], BF16, kind="ExternalInput").ap()
    wa_d = nc.dram_tensor("W_attn", [C, 3 * C], BF16, kind="ExternalInput").ap()
    ba_d = nc.dram_tensor("b_attn", [1, 3 * C], BF16, kind="ExternalInput").ap()
    wp_d = nc.dram_tensor("W_proj", [C, C], BF16, kind="ExternalInput").ap()
    bp_d = nc.dram_tensor("b_proj", [1, C], BF16, kind="ExternalInput").ap()
    y_d = nc.dram_tensor("y", [T, C], BF16, kind="ExternalOutput").ap()

    with tile.TileContext(nc) as tc:
        _emit(nc, tc, x_d, wa_d, ba_d, wp_d, bp_d, y_d, with_bias)
    nc.compile()
    return nc


def _pump(g, n=1):
    if g is None:
        return
    for _ in range(n):
        try:
            next(g)
        except StopIteration:
            return


def _drain(g):
    if g is None:
        return
    for _ in g:
        pass


def _emit(nc, tc, x_d, wa_d, ba_d, wp_d, bp_d, y_d, with_bias):
    from contextlib import ExitStack
    from itertools import chain

    ctx = ExitStack()
    with ctx:
        const_pool = ctx.enter_context(tc.tile_pool(name="consts", bufs=1))
        # ps_work holds the merged [128,1024] ST tiles (2 banks each);
        # ps_acc holds 1-bank accumulation tiles (qk/v/y) + OT' tiles.
        ps_work = ctx.enter_context(tc.tile_pool(name="ps_work", bufs=2, space="PSUM"))
        ps_acc = ctx.enter_context(tc.tile_pool(name="ps_acc", bufs=2, space="PSUM"))

        # ---- inputs: merged tiles, need-ordered 3D DMAs on both rings ----
        # scalar ring: xT h0, qk first halves, xT h1, qk second halves
        # sync ring:   v-part (vc-major), then W_proj
        xt_pool = ctx.enter_context(tc.tile_pool(name="xt", bufs=1))
        xT = []
        for cc in range(CC):
            t_ = xt_pool.tile([128, T], BF16, name=f"xT_{cc}", tag=f"xT{cc}")
            xT.append(t_)

        def xts(cc, t0, w):
            return xT[cc][:, t0 : t0 + w]

        # W_all layout: [v vc-major 2*2304 | qh0 2304 | kh0 2304 | qh1 2304
        #                | kh1 2304] where qh/kh h holds head-pairs 3h..3h+2
        w_pool = ctx.enter_context(tc.tile_pool(name="w", bufs=1))
        W_all = w_pool.tile([128, CC * 3 * C], BF16, name="W_all", tag="W")

        def wv(cc, vc):
            base = vc * 2304 + cc * 384
            return W_all[:, base : base + 384]

        def wqk(cc, cp):
            part, hp = cp // CC, cp % CC
            half, hh = hp // 3, hp % 3
            base = 4608 + half * 4608 + part * 2304 + cc * 384 + hh * 128
            return W_all[:, base : base + 128]

        def _w_dma(queue, dst_base, src_col0, width):
            queue.dma_start(
                W_all[:, dst_base : dst_base + CC * width].rearrange(
                    "p (cc w) -> p cc w", cc=CC
                ),
                wa_d[:, src_col0 : src_col0 + width].rearrange(
                    "(cc p) w -> p cc w", p=128
                ),
            )

        # emission order defines ring order: x on the scalar ring, W on sync
        for cc in range(CC):
            nc.scalar.dma_start(
                xT[cc][:, 0:512], x_d[cc * 128 : (cc + 1) * 128, 0:512]
            )
        _w_dma(nc.sync, 0, 1536, 384)      # v vc=0
        _w_dma(nc.sync, 2304, 1920, 384)   # v vc=1
        _w_dma(nc.sync, 4608, 0, 384)      # q half 0 (pairs 0-2)
        _w_dma(nc.sync, 6912, 768, 384)    # k half 0
        for cc in range(CC):
            nc.scalar.dma_start(
                xT[cc][:, 512:1024], x_d[cc * 128 : (cc + 1) * 128, 512:1024]
            )
        _w_dma(nc.sync, 9216, 384, 384)    # q half 1 (pairs 3-5)
        _w_dma(nc.sync, 11520, 1152, 384)  # k half 1

        wp_pool = ctx.enter_context(tc.tile_pool(name="wp", bufs=1))
        Wp_all = wp_pool.tile([128, CC * C], BF16, name="Wp_all", tag="Wp")
        nc.sync.dma_start(
            Wp_all.rearrange("p (cc w) -> p cc w", cc=CC),
            wp_d.rearrange("(cc p) w -> p cc w", p=128),
        )

        def wpj(cc, oc):
            return Wp_all[:, cc * C + oc * 384 : cc * C + (oc + 1) * 384]

        # ---- constants -------------------------------------------------
        # tri[j, i] = 1.0 if j <= i else 0.0   (keep lower-causal in [j,i] layout)
        tri_f32 = const_pool.tile([128, 128], F32, name="tri_f32")
        nc.gpsimd.memset(tri_f32[:], 1.0)
        nc.gpsimd.affine_select(
            out=tri_f32[:], in_=tri_f32[:], compare_op=ALU.is_ge, fill=0.0,
            base=0, pattern=[[1, 128]], channel_multiplier=-1,
        )
        tri = const_pool.tile([128, 128], BF16, name="tri")
        nc.vector.tensor_copy(tri[:], tri_f32[:])
        ones32 = const_pool.tile([128, 16], F32, name="ones32")
        nc.gpsimd.memset(ones32[:], 1.0)

        # warm the exp table set early (hidden under input DMA)
        expwarm = const_pool.tile([1, 1], F32, name="expwarm")
        nc.scalar.activation(expwarm[:], ones32[0:1, 0:1], AF.Exp)

        if with_bias:
            ones_row = const_pool.tile([1, 512], BF16, name="ones_row")
            nc.gpsimd.memset(ones_row[:], 1.0)
            ba_sb = const_pool.tile([1, 3 * C], BF16, name="ba_sb")
            nc.scalar.dma_start(ba_sb[:], ba_d[:, :])
            bp_sb = const_pool.tile([1, C], BF16, name="bp_sb")
            nc.scalar.dma_start(bp_sb[:], bp_d[:, :])
            # column layout of b_attn qk-part for per-partition bias add
            ba_col = const_pool.tile([128, QKCP], F32, name="ba_col")
            ba_colb = const_pool.tile([128, QKCP], BF16, name="ba_colb")
            nc.scalar.dma_start(
                ba_colb[:],
                ba_d[:, 0 : QKCP * 128].rearrange("a (cp p) -> (a p) cp", p=128),
            )
            nc.vector.tensor_copy(ba_col[:], ba_colb[:])

        # ---- phase B: Vp then qkT (attention needs all Vp) -------------
        vp_pool = ctx.enter_context(tc.tile_pool(name="vp", bufs=1))
        Vp = []
        for tp in range(TP):
            t_ = vp_pool.tile([128, H * 65], BF16, name=f"Vp_{tp}", tag=f"Vp{tp}")
            Vp.append(t_)
            nc.vector.tensor_copy(
                t_.rearrange("p (h e) -> p h e", e=65)[:, :, 64:65],
                ones32[:, 0:H].rearrange("p (h e) -> p h e", e=1),
            )

        def v_chunk_gen(tp):
            # each yield leaves a consistent state: the chain's Vp copy is
            # emitted before the final yield so consumers can follow safely
            for vc in range(2):  # v cols [1536+384*vc : 1536+384*(vc+1)]
                pv = ps_acc.tile([128, 384], F32, name=f"ps_v_{vc}_{tp}", tag="acc")
                for cc in range(CC):
                    nc.tensor.matmul(
                        pv[:],
                        xts(cc, tp * 128, 128),
                        wv(cc, vc),
                        start=(cc == 0),
                        stop=(cc == CC - 1) and not with_bias,
                    )
                    if cc < CC - 1:
                        yield
                if with_bias:
                    nc.tensor.matmul(
                        pv[:],
                        ones_row[:, 0:128],
                        ba_sb[:, 1536 + vc * 384 : 1536 + (vc + 1) * 384],
                        start=False,
                        stop=True,
                    )
                # one strided copy drops all 6 heads into their Vp slots;
                # ScalarE carries it (idle during the projection ramp)
                nc.scalar.copy(
                    Vp[tp].rearrange("p (h e) -> p h e", e=65)[
                        :, 6 * vc : 6 * vc + 6, 0:64
                    ],
                    pv.rearrange("p (h e) -> p h e", e=64)[:, :, :],
                )
                yield

        qkt_pool = ctx.enter_context(tc.tile_pool(name="qkt", bufs=1))
        qkT = []
        for cp in range(QKCP):
            t_ = qkt_pool.tile([128, T], BF16, name=f"qkT_{cp}", tag=f"qkT{cp}")
            qkT.append(t_)

        def qk_pair_gen(hp):
            # copy emitted before the chain's final yield (see v_chunk_gen)
            for tb in range(TB):
                for cp in (hp, 6 + hp):
                    pq = ps_acc.tile([128, 512], F32, name=f"ps_qk_{cp}_{tb}", tag="acc")
                    for cc in range(CC):
                        nc.tensor.matmul(
                            pq[:],
                            wqk(cc, cp),
                            xts(cc, tb * 512, 512),
                            start=(cc == 0),
                            stop=(cc == CC - 1),
                        )
                        if cc < CC - 1:
                            yield
                    if with_bias:
                        # b_attn[c'] folded in as a per-partition scalar add
                        nc.vector.tensor_scalar_add(
                            qkT[cp][:, tb * 512 : (tb + 1) * 512],
                            pq[:],
                            ba_col[:, cp : cp + 1],
                        )
                    else:
                        nc.vector.tensor_copy(
                            qkT[cp][:, tb * 512 : (tb + 1) * 512], pq[:]
                        )
                    yield

        # ---- phase C/D: attention (ib-major) interleaved with proj -----
        atn_pool = ctx.enter_context(tc.tile_pool(name="atn", bufs=1))
        ATn = []
        for cp in range(CC):
            t_ = atn_pool.tile([128, T], BF16, name=f"ATn_{cp}", tag=f"ATn{cp}")
            ATn.append(t_)

        est_pool = ctx.enter_context(tc.tile_pool(name="est", bufs=10))
        nrm_pool = ctx.enter_context(tc.tile_pool(name="nrm", bufs=4))
        y_pool = ctx.enter_context(tc.tile_pool(name="ysb", bufs=2))

        def norm(hp, ib, po, i0, w):
            """Normalize OT'->ATn for columns [i0, i0+w) of i-block ib.

            Batched over both heads: one scatter/recip/gather/broadcast chain.
            """
            key = f"{hp}_{ib}_{i0}"
            otu = nrm_pool.tile([65, 2 * w], BF16, name=f"otu_{key}", tag="otu")
            for s in range(2):
                nc.vector.tensor_copy(otu[:, s * w : (s + 1) * w], po[s][:, i0 : i0 + w])
            nw = 2 * w // 128
            zs = nrm_pool.tile([128, nw], BF16, name=f"zs_{key}", tag="zs")
            nc.sync.dma_start(zs[:], otu[64:65, :])
            zr = nrm_pool.tile([128, nw], F32, name=f"zr_{key}", tag="zr")
            nc.vector.reciprocal(zr[:], zs[:])
            zrb = nrm_pool.tile([128, nw], BF16, name=f"zrb_{key}", tag="zrb")
            nc.vector.tensor_copy(zrb[:], zr[:])
            zinv = nrm_pool.tile([1, 2 * w], BF16, name=f"zinv_{key}", tag="zinv")
            nc.sync.dma_start(zinv[:], zrb[:])
            zb = nrm_pool.tile([64, 2 * w], BF16, name=f"zb_{key}", tag="zb")
            nc.gpsimd.partition_broadcast(zb[:], zinv[:])
            for s in range(2):
                nc.vector.tensor_tensor(
                    ATn[hp][64 * s : 64 * s + 64, ib * 512 + i0 : ib * 512 + i0 + w],
                    otu[0:64, s * w : (s + 1) * w],
                    zb[:, s * w : (s + 1) * w],
                    op=ALU.mult,
                )

        def attention(hp, ib, quartered=False, filler=None, drain_filler=True):
            qt = qkT[hp]
            kt = qkT[6 + hp]
            po = {}
            for s in range(2):  # head 2*hp + s
                po[s] = ps_acc.tile([65, 512], F32, name=f"ps_ot_{hp}_{ib}_{s}", tag="ot", bufs=2)
            njc = 4 * (ib + 1)
            ests = {}

            def emit_st(jc):
                r = jc - 4 * ib
                col0 = max(r, 0) * 128
                # merged pair tile: head A in cols [0:512], head B in [512:1024]
                pst = ps_work.tile([128, 1024], F32, name=f"ps_st_{hp}_{ib}_{jc}", tag="ps")
                for s in range(2):
                    r0 = 64 * s
                    # row-packed pair: s=0 uses PE rows 0-63, s=1 rows 64-127
                    nc.tensor.matmul(
                        pst[:, 512 * s + col0 : 512 * s + 512],
                        kt[r0 : r0 + 64, jc * 128 : (jc + 1) * 128],
                        qt[r0 : r0 + 64, ib * 512 + col0 : (ib + 1) * 512],
                        start=True,
                        stop=True,
                    )
                est = est_pool.tile([128, 1024], BF16, name=f"est_{hp}_{ib}_{jc}", tag="est")
                ests[jc] = est
                nc.scalar.activation(
                    est.rearrange("p (a f) -> p a f", a=2)[:, :, col0:512],
                    pst.rearrange("p (a f) -> p a f", a=2)[:, :, col0:512],
                    AF.Exp,
                    scale=SCALE,
                )
                if r >= 0:
                    for s in range(2):
                        # mask the diagonal 128x128 sub-block (multiplicative)
                        nc.vector.tensor_tensor(
                            est[:, 512 * s + col0 : 512 * s + col0 + 128],
                            est[:, 512 * s + col0 : 512 * s + col0 + 128],
                            tri[:],
                            op=ALU.mult,
                        )

            def emit_pv(jc):
                r = jc - 4 * ib
                col0 = max(r, 0) * 128
                est = ests.pop(jc)
                if not quartered:
                    for s in range(2):
                        h = 2 * hp + s
                        nc.tensor.matmul(
                            po[s][:, col0:512],
                            Vp[jc][:, h * 65 : h * 65 + 65],
                            est[:, 512 * s + col0 : 512 * s + 512],
                            start=(jc == 0),
                            stop=(jc == njc - 1),
                        )
                else:
                    # per-quarter accumulation stop: quarter q (cols q*128..)
                    # completes at jc == 4+q, freeing its normalization early.
                    # start only once per PSUM bank (zeroes the whole region).
                    for s in range(2):
                        h = 2 * hp + s
                        for q in range(max(r, 0), 4):
                            nc.tensor.matmul(
                                po[s][:, q * 128 : (q + 1) * 128],
                                Vp[jc][:, h * 65 : h * 65 + 65],
                                est[:, 512 * s + q * 128 : 512 * s + (q + 1) * 128],
                                start=(jc == 0 and q == 0),
                                stop=(jc == 4 + q),
                            )
                    if r >= 0:
                        # quarter r just received its stop -> normalize it
                        norm(hp, ib, po, r * 128, 128)

            # software pipeline: ST(jc) ahead of PV(jc-1), filler in between
            for jc in range(njc):
                emit_st(jc)
                _pump(filler, 1)
                if jc >= 1:
                    emit_pv(jc - 1)
                    _pump(filler, 1)
            emit_pv(njc - 1)
            if not quartered:
                norm(hp, ib, po, 0, 512)
            if drain_filler:
                _drain(filler)

        def proj_gen(tp, cps=range(CC), py_held=None, start=True, stop=True):
            """Emit proj-chain matmuls for t-block tp over contraction tiles
            cps.  With py_held, continues previously-started chains."""
            pys = py_held if py_held is not None else {}
            y_sb = None
            if stop:
                y_sb = y_pool.tile([128, C], BF16, name=f"y_sb_{tp}", tag="y_sb")
            for oc in range(2):
                if oc not in pys:
                    pys[oc] = ps_acc.tile(
                        [128, 384], F32, name=f"ps_y_{tp}_{oc}", tag="acc"
                    )
                py = pys[oc]
                cps_l = list(cps)
                for k, cp in enumerate(cps_l):
                    nc.tensor.matmul(
                        py[:],
                        ATn[cp][:, tp * 128 : (tp + 1) * 128],
                        wpj(cp, oc),
                        start=start and (k == 0),
                        stop=stop and (k == len(cps_l) - 1) and not with_bias,
                    )
                    yield
                if stop and with_bias:
                    nc.tensor.matmul(
                        py[:],
                        ones_row[:, 0:128],
                        bp_sb[:, oc * 384 : (oc + 1) * 384],
                        start=False,
                        stop=True,
                    )
                    yield
                if stop:
                    # copy+DMA per half so the first half streams out while
                    # the second chain finishes
                    nc.vector.tensor_copy(
                        y_sb[:, oc * 384 : (oc + 1) * 384], pys[oc][:]
                    )
                    nc.scalar.dma_start(
                        y_d[tp * 128 : (tp + 1) * 128, oc * 384 : (oc + 1) * 384],
                        y_sb[:, oc * 384 : (oc + 1) * 384],
                    )

        # ---- emission schedule -----------------------------------------
        # Each head pair runs ib=0 then ib=1 back-to-back, fed by the
        # remaining qk/v projection work as fillers.  All eight proj tiles
        # are deferred to the tail, forming a dense PE block that hides the
        # last attention's exp chain and keeps the PE clock warm.
        for tp in range(4):
            _drain(v_chunk_gen(tp))
        _drain(qk_pair_gen(0))
        attention(
            0, 0,
            filler=chain(*[v_chunk_gen(tp) for tp in range(4, 8)]),
        )
        # rolling filler: qk pair hp+1 spreads across attention(hp,0) and
        # (hp,1); its tb=0 half is forced out before attention(hp+1,0) STs
        g = {hp: qk_pair_gen(hp) for hp in range(1, 6)}
        _pump(g[1], 12)
        attention(0, 1, filler=g[1])
        for hp in range(1, 5):
            attention(hp, 0, filler=g[hp + 1], drain_filler=False)
            _pump(g[hp + 1], 12)  # force out the tb=0 remainder
            attention(hp, 1, filler=g[hp + 1])
        # proj(0) cp 0..4 don't touch ATn[5]: they fill attention(5,0)
        py0 = {}
        attention(5, 0, filler=proj_gen(0, range(5), py0, True, False))
        attention(
            5, 1, quartered=True,
            filler=chain(
                proj_gen(0, [5], py0, False, True),
                proj_gen(1), proj_gen(2), proj_gen(3),
            ),
        )
        # tail: per-quarter norms pipelined during the loop feed these chains
        for tp in range(4, 8):
            _drain(proj_gen(tp))


def kernel(x, W_attn, b_attn, W_proj, b_proj, _trace=False, _trace_kwargs=None):
    import ml_dtypes

    bf16 = ml_dtypes.bfloat16
    with_bias = bool(np.any(np.asarray(b_attn)) or np.any(np.asarray(b_proj)))
    x = np.ascontiguousarray(np.asarray(x).astype(bf16))
    W_attn = np.ascontiguousarray(np.asarray(W_attn).astype(bf16))
    b_attn = np.ascontiguousarray(np.asarray(b_attn).astype(bf16)).reshape(1, 3 * C)
    W_proj = np.ascontiguousarray(np.asarray(W_proj).astype(bf16))
    b_proj = np.ascontiguousarray(np.asarray(b_proj).astype(bf16)).reshape(1, C)

    key = f"prog_{with_bias}"
    if key not in _PROGRAM_CACHE:
        _PROGRAM_CACHE[key] = build_program(with_bias)
    nc = _PROGRAM_CACHE[key]

    in_maps = [
        {
            "xT": np.ascontiguousarray(x[b].T),
            "W_attn": W_attn,
            "b_attn": b_attn,
            "W_proj": W_proj,
            "b_proj": b_proj,
        }
        for b in range(NCORES)
    ]
    res = run_bass_kernel_spmd(
        nc,
        in_maps,
        core_ids=list(range(NCORES)),
        trace=_trace,
        **(_trace_kwargs or {}),
    )
    out = np.stack(
        [res.results[b]["y"].astype(np.float32) for b in range(NCORES)], axis=0
    )
    if _trace:
        return out, res
    return out


if __name__ == "__main__":
    rng = np.random.default_rng(0)
    x = rng.standard_normal((NCORES, T, C)).astype(np.float32)
    W_attn = (rng.standard_normal((C, 3 * C)) * 0.02).astype(np.float32)
    b_attn = np.zeros(3 * C, np.float32)
    W_proj = (rng.standard_normal((C, C)) * 0.02).astype(np.float32)
    b_proj = np.zeros(C, np.float32)
    y = kernel(x=x, W_attn=W_attn, b_attn=b_attn, W_proj=W_proj, b_proj=b_proj)
    print("out", y.shape, y.dtype, np.abs(y).max())
